# revision 2
# baseline (speedup 1.0000x reference)
"""Multi-head attention (B=4, S=1024, H=1024, heads=16) on 8 trn2 NeuronCores.

Sharding: data-parallel over batch (4) x tensor-parallel over head-groups (2).
Core c handles batch c//2, heads [8*(c%2), 8*(c%2)+8).

Per-core kernel:
  - q/k projections in bf16 (fp32 PSUM), bias-added and cast to fp8 e4m3,
    then folded (SBUF->SBUF DMA) into [32, 2(t), S] per head so the scores
    matmul can run in fp8 DoubleRow mode (2 contraction rows/cycle).
  - scores s_T[j, i] accumulate per (head, j-block); exp runs on ScalarE with
    scale=1/8 (the attention scale), multiplied by host-precomputed
    exp(attn_bias)^T on VectorE/GpSimdE.
  - ctx_T plus the softmax denominator come from one bf16 matmul per
    (head, jb, i-half): stationary = [vh | ones] (65 cols), PSUM-accumulated
    over j-blocks.
  - normalization: the raw denominator row is copied to SBUF (bf16),
    partition-broadcast with a ones-column matmul into PSUM, reciprocal'd
    (custom DVE), then multiplied into ctxn.
  - output projection is row-parallel bf16; partial results stored bf16 and
    summed (plus bo) on the host.

Scheduling: only pair 0's q/k projection runs up front. The v projection and
pairs 1-3's projections are emitted as small chunks interleaved into the
attention block stream, so ScalarE's exp (the attention-phase pacer) starts
~10us into the kernel and every engine stays busy.
"""

import numpy as np
import ml_dtypes

BF16 = ml_dtypes.bfloat16
F8E4 = ml_dtypes.float8_e4m3

S = 1024
HID = 1024
GCOL = 512  # hidden cols per core (8 heads * 64)
DH = 64
P = 128
NPAIR = 4  # head pairs per core
NJB = 8  # key blocks of 128
NCB = 8  # contraction blocks of 128
NIB = 8  # query blocks of 128

CTX_LAG = 8  # blocks the ctx matmuls trail the scores stream
E_BUFS = 12
EB_BUFS = 12

_CACHED_NC = None


def _build_nc():
    import concourse.bass as bass
    import concourse.mybir as mybir
    import concourse.tile as tile
    from concourse import bacc
    from contextlib import ExitStack

    f32 = mybir.dt.float32
    bf16 = mybir.dt.bfloat16
    f8 = mybir.dt.float8e4
    AF = mybir.ActivationFunctionType
    PM = mybir.MatmulPerfMode

    nc = bacc.Bacc(
        "TRN2",
        target_bir_lowering=False,
        debug=False,
        enable_asserts=False,
        num_devices=8,
    )

    qT = nc.dram_tensor("qT", [HID, S], bf16, kind="ExternalInput").ap()
    kT = nc.dram_tensor("kT", [HID, S], bf16, kind="ExternalInput").ap()
    vT = nc.dram_tensor("vT", [HID, S], bf16, kind="ExternalInput").ap()
    wq = nc.dram_tensor("wq", [HID, GCOL], bf16, kind="ExternalInput").ap()
    wk = nc.dram_tensor("wk", [HID, GCOL], bf16, kind="ExternalInput").ap()
    wv = nc.dram_tensor("wv", [HID, GCOL], bf16, kind="ExternalInput").ap()
    wo = nc.dram_tensor("wo", [GCOL, HID], bf16, kind="ExternalInput").ap()
    bq = nc.dram_tensor("bq", [GCOL], f32, kind="ExternalInput").ap()
    bk = nc.dram_tensor("bk", [GCOL], f32, kind="ExternalInput").ap()
    bv = nc.dram_tensor("bv", [GCOL], bf16, kind="ExternalInput").ap()
    expb = nc.dram_tensor("expb", [8, S, S], bf16, kind="ExternalInput").ap()
    out = nc.dram_tensor("out", [S, HID], bf16, kind="ExternalOutput").ap()

    with tile.TileContext(nc) as tc, ExitStack() as ctx:
        const = ctx.enter_context(tc.tile_pool(name="const", bufs=1))
        inT = ctx.enter_context(tc.tile_pool(name="inT", bufs=24))
        proj = ctx.enter_context(tc.tile_pool(name="proj", bufs=1))
        work = ctx.enter_context(tc.tile_pool(name="work", bufs=2))
        psum = ctx.enter_context(tc.tile_pool(name="psum", bufs=2, space="PSUM"))

        # ---- constants / weights ----
        wq_sb = const.tile([P, NCB, GCOL], bf16, tag="wq")
        wk_sb = const.tile([P, NCB, GCOL], bf16, tag="wk")
        wv_sb = const.tile([P, NCB, GCOL], bf16, tag="wv")
        wo_sb = const.tile([P, NPAIR, HID], bf16, tag="wo")
        wq_r = wq.rearrange("(cb p) n -> p cb n", p=P)
        wk_r = wk.rearrange("(cb p) n -> p cb n", p=P)
        wv_r = wv.rearrange("(cb p) n -> p cb n", p=P)
        bq_sb = const.tile([P, NPAIR], f32, tag="bq")
        bk_sb = const.tile([P, NPAIR], f32, tag="bk")
        nc.sync.dma_start(out=bq_sb, in_=bq.rearrange("(pr p) -> p pr", p=P))
        nc.sync.dma_start(out=bk_sb, in_=bk.rearrange("(pr p) -> p pr", p=P))
        bv_sb = const.tile([1, GCOL], bf16, tag="bv")
        nc.sync.dma_start(out=bv_sb, in_=bv.rearrange("(a n) -> a n", a=1))
        ones_k1 = const.tile([1, P], bf16, tag="ones_k1")
        nc.vector.memset(ones_k1, 1.0)
        ones_bc = const.tile([1, DH], bf16, tag="ones_bc")
        nc.vector.memset(ones_bc, 1.0)
        wup_sb = const.tile([1, GCOL], bf16, tag="wup")
        nc.vector.memset(wup_sb, 1.0)
        gp_w = const.tile([1, GCOL], bf16, tag="gp_w")
        nc.gpsimd.memset(gp_w, 1.0)

        # fp8 folded q/k: [32, hl, t, S]; contraction pair (k, t) maps
        # d = hl*64 + t*32 + k so DoubleRow contracts a full head dim.
        qfold = [proj.tile([32, 2, 2, S], f8, name=f"qf8_{p}", tag=f"qf8_{p}")
                 for p in range(NPAIR)]
        kfold = [proj.tile([32, 2, 2, S], f8, name=f"kf8_{p}", tag=f"kf8_{p}")
                 for p in range(NPAIR)]
        # vh_sb[jb]: [j in block, head, 65] where col 64 is ones (denominator)
        vh_sb = [proj.tile([P, 8, DH + 1], bf16, name=f"vh{i}", tag=f"vh{i}")
                 for i in range(NJB)]
        ctxn = [proj.tile([P, S], bf16, name=f"ctxn{i}", tag=f"ctxn{i}")
                for i in range(NPAIR)]

        # ---- PE/ACT warmup during initial DMA wait ----
        for w in range(8):
            wp = psum.tile([P, GCOL], f32, name=f"wup{w}", tag="mm")
            nc.tensor.matmul(wp, lhsT=ones_k1, rhs=wup_sb, start=True, stop=True)
            if w == 7:
                es_w = work.tile([P, S], bf16, name="es_w", tag="es", bufs=3)
                nc.scalar.activation(es_w[:, 0:GCOL], wp, AF.Exp, scale=0.125)
        rbc_w = work.tile([DH, GCOL], f32, name="rbc_w", tag="rbc", bufs=2)
        nc.vector.memset(rbc_w, 1.0)
        nc.vector.reciprocal_approx_fast(rbc_w, rbc_w)

        # ---- q/k input + weight loads ----
        qk_tiles = {}
        for tname, src, w_r, w_sb in (("q", qT, wq_r, wq_sb), ("k", kT, wk_r, wk_sb)):
            tl = []
            for cb in range(NCB):
                nc.sync.dma_start(out=w_sb[:, cb, :], in_=w_r[:, cb, :])
                t = inT.tile([P, S], bf16, name=f"{tname}T{cb}", tag="inT")
                nc.sync.dma_start(out=t, in_=src[cb * P:(cb + 1) * P, :])
                tl.append(t)
            qk_tiles[tname] = tl

        qf_name = {"q": (qfold, bq_sb, "q"), "k": (kfold, bk_sb, "k")}

        def qk_half(tname, pr, ic):
            """One half-projection chunk: 8 matmuls + bias-add/f8-cast."""
            fold_l, b_sb, nm = qf_name[tname]
            w_sb = wq_sb if tname == "q" else wk_sb
            pp = psum.tile([P, GCOL], f32, name=f"pp{nm}{pr}_{ic}", tag="mm")
            for cb in range(NCB):
                nc.tensor.matmul(
                    pp,
                    lhsT=w_sb[:, cb, pr * P:(pr + 1) * P],
                    rhs=qk_tiles[tname][cb][:, ic * 512:(ic + 1) * 512],
                    start=(cb == 0),
                    stop=(cb == NCB - 1),
                )
            qf = work.tile([P, GCOL], f8, name=f"qf{nm}{pr}_{ic}", tag=f"{nm}f",
                           bufs=2)
            nc.vector.tensor_scalar_add(qf, pp, b_sb[:, pr:pr + 1])
            # fold this i-half into [32, hl, t, i] (fp8 scores layout)
            for hl in range(2):
                for t in range(2):
                    base = hl * DH + t * 32
                    nc.gpsimd.dma_start(
                        out=fold_l[pr][0:32, hl, t, ic * 512:(ic + 1) * 512],
                        in_=qf[base:base + 32, :],
                    )

        def v_chunk(jb):
            """One v-projection chunk: project j-block jb for all 8 heads."""
            ps = psum.tile([P, GCOL], f32, name=f"vp{jb}", tag="mm")
            for cb in range(NCB):
                nc.tensor.matmul(
                    ps,
                    lhsT=vtiles[cb][:, jb * P:(jb + 1) * P],
                    rhs=wv_sb[:, cb, :],
                    start=(cb == 0),
                    stop=False,
                )
            nc.tensor.matmul(ps, lhsT=ones_k1, rhs=bv_sb, start=False, stop=True)
            nc.vector.tensor_copy(
                out=vh_sb[jb][:, :, 0:DH],
                in_=ps.rearrange("p (h d) -> p h d", d=DH),
            )
            nc.vector.memset(vh_sb[jb][:, :, DH:DH + 1], 1.0)

        # ---- pair 0's q/k projection up front ----
        for tname in ("q", "k"):
            for ic in range(2):
                qk_half(tname, 0, ic)

        # ---- v/wo loads (matmuls drain later as chunks) ----
        vtiles = []
        for cb in range(NCB):
            nc.sync.dma_start(out=wv_sb[:, cb, :], in_=wv_r[:, cb, :])
            t = inT.tile([P, S], bf16, name=f"vT{cb}", tag="inT")
            nc.sync.dma_start(out=t, in_=vT[cb * P:(cb + 1) * P, :])
            vtiles.append(t)
        for pr in range(NPAIR):
            nc.sync.dma_start(
                out=wo_sb[:, pr, :],
                in_=wo.rearrange("(pr p) n -> p pr n", p=P)[:, pr, :],
            )

        # ---- chunk drain plan: global block index -> emitters ----
        drain_plan = {}
        for jb in range(NJB):
            drain_plan.setdefault(jb, []).append(lambda jb=jb: v_chunk(jb))
        slot = {1: [8, 9, 10, 11], 2: [22, 24, 26, 28], 3: [38, 40, 42, 44]}
        for pr in (1, 2, 3):
            for i, (tname, ic) in enumerate(
                (t, c) for t in ("q", "k") for c in range(2)
            ):
                drain_plan.setdefault(slot[pr][i], []).append(
                    lambda t=tname, pr=pr, ic=ic: qk_half(t, pr, ic)
                )

        # ---- normalize ----
        def normalize_head(pr, hl, ic, cr):
            """ctxn[pr] rows for head hl <- cr/r via matmul partition-bcast."""
            r_sb = work.tile([1, GCOL], bf16, name=f"r{pr}{hl}{ic}", tag="rrow",
                             bufs=2)
            nc.vector.tensor_copy(r_sb, cr[(hl, ic)][DH:DH + 1, :])
            rbcp = psum.tile([DH, GCOL], f32, name=f"rp{pr}{hl}{ic}", tag="mm")
            nc.tensor.matmul(rbcp, lhsT=ones_bc, rhs=r_sb, start=True, stop=True)
            rbc = work.tile([DH, GCOL], f32, name=f"rb{pr}{hl}{ic}", tag="rbc",
                            bufs=2)
            nc.vector.reciprocal_approx_fast(rbc, rbcp)
            if hl == 0:
                nc.vector.tensor_mul(
                    ctxn[pr][0:DH, ic * 512:(ic + 1) * 512],
                    cr[(hl, ic)][0:DH, :],
                    rbc,
                )
            else:
                ch = work.tile([DH, GCOL], bf16, name=f"ch{pr}{hl}{ic}", tag="ch",
                               bufs=2)
                nc.vector.tensor_mul(ch, cr[(hl, ic)][0:DH, :], rbc)
                nc.gpsimd.dma_start(
                    out=ctxn[pr][DH:2 * DH, ic * 512:(ic + 1) * 512], in_=ch
                )

        # ---- attention ----
        block_idx = [0]

        def attention_pair(pr, pending_norm):
            cr = {}
            cr_queue = []

            def emit_cr(jb, hl, e):
                h = 2 * pr + hl
                if (hl, 0) not in cr:
                    for chl in range(2):
                        for cic in range(2):
                            cr[(chl, cic)] = psum.tile(
                                [DH + 1, GCOL], f32,
                                name=f"cr{pr}_{chl}_{cic}", tag="cr", bufs=4,
                            )
                for ic in range(2):
                    nc.tensor.matmul(
                        cr[(hl, ic)],
                        lhsT=vh_sb[jb][:, h, :],
                        rhs=e[:, ic * 512:(ic + 1) * 512],
                        start=(jb == 0),
                        stop=(jb == NJB - 1),
                    )

            for jb in range(NJB):
                for hl in range(2):
                    bi = block_idx[0]
                    h = 2 * pr + hl
                    eb = work.tile([P, S], bf16, name=f"eb{h}_{jb}", tag="eb",
                                   bufs=EB_BUFS)
                    nc.sync.dma_start(out=eb, in_=expb[h, jb * P:(jb + 1) * P, :])
                    s_ps = psum.tile([P, S], f32, name=f"s{h}_{jb}", tag="mm")
                    for ic in range(2):
                        nc.tensor.matmul(
                            s_ps[:, ic * 512:(ic + 1) * 512],
                            lhsT=kfold[pr][0:32, hl, :, jb * P:(jb + 1) * P],
                            rhs=qfold[pr][0:32, hl, :, ic * 512:(ic + 1) * 512],
                            start=True,
                            stop=True,
                            perf_mode=PM.DoubleRow,
                        )
                    es = work.tile([P, S], bf16, name=f"es{h}_{jb}", tag="es",
                                   bufs=3)
                    nc.scalar.activation(es, s_ps, AF.Exp, scale=0.125)
                    e = work.tile([P, S], bf16, name=f"e{h}_{jb}", tag="e",
                                  bufs=E_BUFS)
                    if bi % 4 == 2:
                        nc.gpsimd.tensor_mul(e, es, eb)
                    else:
                        nc.vector.tensor_mul(e, es, eb)
                    cr_queue.append((jb, hl, e))
                    if len(cr_queue) > CTX_LAG:
                        emit_cr(*cr_queue.pop(0))
                    for fn in drain_plan.pop(bi, ()):
                        fn()
                    block_idx[0] += 1
                if jb == 1 and pending_norm:
                    for fn in pending_norm[:2]:
                        fn()
                    pending_norm = pending_norm[2:]
                if jb == 2 and pending_norm:
                    for fn in pending_norm:
                        fn()
                    pending_norm = None
            for item in cr_queue:
                emit_cr(*item)
            return [
                (lambda hl=hl, ic=ic: normalize_head(pr, hl, ic, cr))
                for ic in range(2)
                for hl in (1, 0)
            ]

        pending = None
        for pr in range(NPAIR):
            pending = attention_pair(pr, pending)

        # ---- output projection (pr3 norm interleaved: ic0, ib0-3, ic1) ----
        def outproj(ib):
            yp = psum.tile([P, HID], f32, name=f"yp{ib}", tag="mm")
            for pr in range(NPAIR):
                for cc in range(2):
                    nc.tensor.matmul(
                        yp[:, cc * 512:(cc + 1) * 512],
                        lhsT=ctxn[pr][:, ib * P:(ib + 1) * P],
                        rhs=wo_sb[:, pr, cc * 512:(cc + 1) * 512],
                        start=(pr == 0),
                        stop=(pr == NPAIR - 1),
                    )
            y_sb = work.tile([P, HID], bf16, name=f"y{ib}", tag="y", bufs=2)
            nc.scalar.activation(y_sb, yp, AF.Copy)
            nc.sync.dma_start(out=out[ib * P:(ib + 1) * P, :], in_=y_sb)

        norm_ic0, norm_ic1 = pending[:2], pending[2:]
        for fn in norm_ic0:
            fn()
        for ib in range(4):
            outproj(ib)
            if ib == 0:
                for fn in norm_ic1:
                    fn()
        for ib in range(4, NIB):
            outproj(ib)

    nc.compile()
    return nc


def _get_nc():
    global _CACHED_NC
    if _CACHED_NC is None:
        _CACHED_NC = _build_nc()
    return _CACHED_NC


def make_in_maps(q, k, v, attn_bias, Wq, Wk, Wv, Wo, bq, bk, bv, bo):
    in_maps = []
    for core in range(8):
        b, g = divmod(core, 2)
        gs = slice(g * GCOL, (g + 1) * GCOL)
        in_maps.append({
            "qT": np.ascontiguousarray(q[b].T).astype(BF16),
            "kT": np.ascontiguousarray(k[b].T).astype(BF16),
            "vT": np.ascontiguousarray(v[b].T).astype(BF16),
            "wq": np.ascontiguousarray(Wq[:, gs]).astype(BF16),
            "wk": np.ascontiguousarray(Wk[:, gs]).astype(BF16),
            "wv": np.ascontiguousarray(Wv[:, gs]).astype(BF16),
            "wo": np.ascontiguousarray(Wo[gs, :]).astype(BF16),
            "bq": np.ascontiguousarray(bq[gs]).astype(np.float32),
            "bk": np.ascontiguousarray(bk[gs]).astype(np.float32),
            "bv": np.ascontiguousarray(bv[gs]).astype(BF16),
            "expb": np.exp(
                attn_bias[b, g * 8:(g + 1) * 8].transpose(0, 2, 1)
            ).astype(BF16),
        })
    return in_maps


def kernel(q, k, v, attn_bias, Wq, Wk, Wv, Wo, bq, bk, bv, bo, _trace=False):
    from concourse.bass_utils import run_bass_kernel_spmd

    args = [np.asarray(x, dtype=np.float32) for x in
            (q, k, v, attn_bias, Wq, Wk, Wv, Wo, bq, bk, bv, bo)]
    q, k, v, attn_bias, Wq, Wk, Wv, Wo, bq, bk, bv, bo = args
    nc = _get_nc()
    in_maps = make_in_maps(q, k, v, attn_bias, Wq, Wk, Wv, Wo, bq, bk, bv, bo)
    res = run_bass_kernel_spmd(nc, in_maps, core_ids=list(range(8)), trace=_trace)
    y = np.zeros((4, S, HID), np.float32)
    for core in range(8):
        y[core // 2] += res.results[core]["out"].astype(np.float32)
    y += bo
    if _trace:
        kernel.last_results = res
    return y


# revision 3
# speedup vs baseline: 1.1306x; 1.1306x over previous
"""Multi-head attention (B=4, S=1024, H=1024, heads=16) on 8 trn2 NeuronCores.

Sharding: data-parallel over batch (4) x tensor-parallel over head-groups (2).
Core c handles batch c//2, heads [8*(c%2), 8*(c%2)+8).

Per-core kernel (all matmuls bf16 with fp32 PSUM accumulation):
  - q/k projections produce qhT/khT in [d, i] layout (head dim on partitions);
    the attention scale 1/8 is applied by the exp activation (scale=0.125).
  - scores computed transposed (keys on partitions): s_T = khT-chunks @ qhT,
    exp on ScalarE, multiplied by host-precomputed exp(attn_bias)^T on
    VectorE (3 of 4 blocks) / GpSimdE (1 of 4).
  - ctx_T and the softmax denominator come from ONE matmul per
    (head, i-half): stationary = [vh | ones] (65 cols), accumulated over key
    blocks with a lag behind the scores stream.
  - normalization: pairs 0-2 use a DMA partition-broadcast of the raw
    denominator row (gpsimd queue, latency hidden in the next pair); pair 3
    (latency-critical tail) broadcasts via a ones-column matmul into PSUM.
  - output projection row-parallel bf16; partials stored bf16, summed (+bo)
    on the host.

Scheduling: only pair 0's q/k projection runs up front. Pair 1-3 projections
and the v projection drain as ~1-2us chunks interleaved into the attention
block stream, so ScalarE's exp (the attention-phase pacer, ~70us total)
starts ~18us into the kernel and all engines stay busy. The first 8 expb
block loads are queued ahead of the v input so attention is never
DMA-starved at the start.
"""

import numpy as np
import ml_dtypes

BF16 = ml_dtypes.bfloat16

S = 1024
HID = 1024
GCOL = 512  # hidden cols per core (8 heads * 64)
DH = 64
P = 128
NPAIR = 4  # head pairs per core
NJB = 8  # key blocks of 128
NCB = 8  # contraction blocks of 128
NIB = 8  # query blocks of 128

CTX_LAG = 10  # blocks the ctx matmuls trail the scores stream
E_BUFS = 13
EB_BUFS = 16

_CACHED_NC = None


def _build_nc():
    import concourse.bass as bass
    import concourse.mybir as mybir
    import concourse.tile as tile
    from concourse import bacc
    from contextlib import ExitStack

    f32 = mybir.dt.float32
    bf16 = mybir.dt.bfloat16
    AF = mybir.ActivationFunctionType

    nc = bacc.Bacc(
        "TRN2",
        target_bir_lowering=False,
        debug=False,
        enable_asserts=False,
        num_devices=8,
    )

    qT = nc.dram_tensor("qT", [HID, S], bf16, kind="ExternalInput").ap()
    kT = nc.dram_tensor("kT", [HID, S], bf16, kind="ExternalInput").ap()
    vT = nc.dram_tensor("vT", [HID, S], bf16, kind="ExternalInput").ap()
    wq = nc.dram_tensor("wq", [HID, GCOL], bf16, kind="ExternalInput").ap()
    wk = nc.dram_tensor("wk", [HID, GCOL], bf16, kind="ExternalInput").ap()
    wv = nc.dram_tensor("wv", [HID, GCOL], bf16, kind="ExternalInput").ap()
    wo = nc.dram_tensor("wo", [GCOL, HID], bf16, kind="ExternalInput").ap()
    bq = nc.dram_tensor("bq", [GCOL], f32, kind="ExternalInput").ap()
    bk = nc.dram_tensor("bk", [GCOL], f32, kind="ExternalInput").ap()
    bv = nc.dram_tensor("bv", [GCOL], bf16, kind="ExternalInput").ap()
    expb = nc.dram_tensor("expb", [8, S, S], bf16, kind="ExternalInput").ap()
    out = nc.dram_tensor("out", [S, HID], bf16, kind="ExternalOutput").ap()

    with tile.TileContext(nc) as tc, ExitStack() as ctx:
        const = ctx.enter_context(tc.tile_pool(name="const", bufs=1))
        inT = ctx.enter_context(tc.tile_pool(name="inT", bufs=24))
        proj = ctx.enter_context(tc.tile_pool(name="proj", bufs=1))
        work = ctx.enter_context(tc.tile_pool(name="work", bufs=2))
        psum = ctx.enter_context(tc.tile_pool(name="psum", bufs=2, space="PSUM"))

        # ---- constants / weights ----
        wq_sb = const.tile([P, NCB, GCOL], bf16, tag="wq")
        wk_sb = const.tile([P, NCB, GCOL], bf16, tag="wk")
        wv_sb = const.tile([P, NCB, GCOL], bf16, tag="wv")
        wo_sb = const.tile([P, NPAIR, HID], bf16, tag="wo")
        wq_r = wq.rearrange("(cb p) n -> p cb n", p=P)
        wk_r = wk.rearrange("(cb p) n -> p cb n", p=P)
        wv_r = wv.rearrange("(cb p) n -> p cb n", p=P)
        bq_sb = const.tile([P, NPAIR], f32, tag="bq")
        bk_sb = const.tile([P, NPAIR], f32, tag="bk")
        nc.sync.dma_start(out=bq_sb, in_=bq.rearrange("(pr p) -> p pr", p=P))
        nc.sync.dma_start(out=bk_sb, in_=bk.rearrange("(pr p) -> p pr", p=P))
        bv_sb = const.tile([1, GCOL], bf16, tag="bv")
        nc.sync.dma_start(out=bv_sb, in_=bv.rearrange("(a n) -> a n", a=1))
        ones_k1 = const.tile([1, P], bf16, tag="ones_k1")
        nc.vector.memset(ones_k1, 1.0)
        ones_bc = const.tile([1, DH], bf16, tag="ones_bc")
        nc.vector.memset(ones_bc, 1.0)
        wup_sb = const.tile([1, GCOL], bf16, tag="wup")
        nc.vector.memset(wup_sb, 1.0)
        gp_w = const.tile([1, GCOL], bf16, tag="gp_w")
        nc.gpsimd.memset(gp_w, 1.0)

        qhT = [proj.tile([P, S], bf16, name=f"qhT{i}", tag=f"qhT{i}")
               for i in range(NPAIR)]
        khT = [proj.tile([P, S], bf16, name=f"khT{i}", tag=f"khT{i}")
               for i in range(NPAIR)]
        # vh_sb[jb]: [j in block, head, 65] where col 64 is ones (denominator)
        vh_sb = [proj.tile([P, 8, DH + 1], bf16, name=f"vh{i}", tag=f"vh{i}")
                 for i in range(NJB)]
        ctxn = [proj.tile([P, S], bf16, name=f"ctxn{i}", tag=f"ctxn{i}")
                for i in range(NPAIR)]

        # ---- PE/ACT warmup during initial DMA wait ----
        for w in range(8):
            wp = psum.tile([P, GCOL], f32, name=f"wup{w}", tag="mm")
            nc.tensor.matmul(wp, lhsT=ones_k1, rhs=wup_sb, start=True, stop=True)
            if w == 7:
                es_w = work.tile([P, S], bf16, name="es_w", tag="es", bufs=3)
                nc.scalar.activation(es_w[:, 0:GCOL], wp, AF.Exp, scale=0.125)
        rbc_w = work.tile([DH + 1, GCOL], f32, name="rbc_w", tag="rbc", bufs=2)
        nc.vector.memset(rbc_w, 1.0)
        nc.vector.reciprocal_approx_fast(rbc_w[0:DH, :], rbc_w[0:DH, :])

        # ---- q/k input + weight loads ----
        qk_tiles = {}
        for tname, src, w_r, w_sb in (("q", qT, wq_r, wq_sb), ("k", kT, wk_r, wk_sb)):
            tl = []
            for cb in range(NCB):
                nc.sync.dma_start(out=w_sb[:, cb, :], in_=w_r[:, cb, :])
                t = inT.tile([P, S], bf16, name=f"{tname}T{cb}", tag="inT")
                nc.sync.dma_start(out=t, in_=src[cb * P:(cb + 1) * P, :])
                tl.append(t)
            qk_tiles[tname] = tl

        def qk_half(tname, pr, ic):
            """One half-projection chunk: 8 matmuls + bias-add to bf16."""
            w_sb, b_sb, dst = (
                (wq_sb, bq_sb, qhT) if tname == "q" else (wk_sb, bk_sb, khT)
            )
            pp = psum.tile([P, GCOL], f32, name=f"pp{tname}{pr}_{ic}", tag="mm")
            for cb in range(NCB):
                nc.tensor.matmul(
                    pp,
                    lhsT=w_sb[:, cb, pr * P:(pr + 1) * P],
                    rhs=qk_tiles[tname][cb][:, ic * 512:(ic + 1) * 512],
                    start=(cb == 0),
                    stop=(cb == NCB - 1),
                )
            nc.vector.tensor_scalar_add(
                dst[pr][:, ic * 512:(ic + 1) * 512], pp, b_sb[:, pr:pr + 1]
            )

        def v_half(jb, gh):
            """Project j-block jb for heads [4*gh, 4*gh+4)."""
            gsl = slice(gh * 256, (gh + 1) * 256)
            ps = psum.tile([P, 256], f32, name=f"vp{jb}_{gh}", tag="mm")
            for cb in range(NCB):
                nc.tensor.matmul(
                    ps,
                    lhsT=vtiles[cb][:, jb * P:(jb + 1) * P],
                    rhs=wv_sb[:, cb, gsl],
                    start=(cb == 0),
                    stop=False,
                )
            nc.tensor.matmul(ps, lhsT=ones_k1, rhs=bv_sb[:, gsl],
                             start=False, stop=True)
            nc.vector.tensor_copy(
                out=vh_sb[jb][:, 4 * gh:4 * gh + 4, 0:DH],
                in_=ps.rearrange("p (h d) -> p h d", d=DH),
            )
            if gh == 1:
                nc.vector.memset(vh_sb[jb][:, :, DH:DH + 1], 1.0)

        # ---- pair 0's q/k projection up front ----
        for tname in ("q", "k"):
            for ic in range(2):
                qk_half(tname, 0, ic)

        # ---- first 8 expb block loads jump ahead of the v input ----
        eb_pre = []
        for bi in range(8):
            jb, hl = bi // 2, bi % 2
            eb = work.tile([P, S], bf16, name=f"eb{hl}_{jb}", tag="eb",
                           bufs=EB_BUFS)
            nc.sync.dma_start(out=eb, in_=expb[hl, jb * P:(jb + 1) * P, :])
            eb_pre.append(eb)

        # ---- v/wo loads (matmuls drain later as chunks) ----
        vtiles = []
        for cb in range(NCB):
            nc.sync.dma_start(out=wv_sb[:, cb, :], in_=wv_r[:, cb, :])
            t = inT.tile([P, S], bf16, name=f"vT{cb}", tag="inT")
            nc.sync.dma_start(out=t, in_=vT[cb * P:(cb + 1) * P, :])
            vtiles.append(t)
        for pr in range(NPAIR):
            nc.sync.dma_start(
                out=wo_sb[:, pr, :],
                in_=wo.rearrange("(pr p) n -> p pr n", p=P)[:, pr, :],
            )

        # ---- chunk drain plan: global block index -> emitters ----
        drain_plan = {}
        for i, (tname, ic) in enumerate(
            (t, c) for t in ("q", "k") for c in range(2)
        ):
            drain_plan.setdefault(i, []).append(
                lambda t=tname, ic=ic: qk_half(t, 1, ic)
            )
        for jb in range(NJB):
            for gh in range(2):
                drain_plan.setdefault(5 + jb, []).append(
                    lambda jb=jb, gh=gh: v_half(jb, gh)
                )
        slot = {2: [20, 22, 24, 26], 3: [36, 38, 40, 42]}
        for pr in (2, 3):
            for i, (tname, ic) in enumerate(
                (t, c) for t in ("q", "k") for c in range(2)
            ):
                drain_plan.setdefault(slot[pr][i], []).append(
                    lambda t=tname, pr=pr, ic=ic: qk_half(t, pr, ic)
                )

        # ---- normalize ----
        def normalize_dma(pr, hl, ic, cr):
            """ctxn rows for head hl <- cr/r via gpsimd-queue DMA broadcast."""
            rbc = work.tile([DH + 1, GCOL], f32, name=f"rbc{pr}{hl}_{ic}",
                            tag="rbc", bufs=2)
            nc.vector.tensor_copy(rbc[DH:DH + 1, :], cr[(hl, ic)][DH:DH + 1, :])
            row = rbc[DH:DH + 1, :]
            row8 = bass.AP(
                tensor=row.tensor,
                offset=row.offset,
                ap=[list(row.ap[0]), [0, 8]] + [list(d) for d in row.ap[1:]],
            )
            nc.gpsimd.dma_start(out=rbc[0:8, :], in_=row8)
            blk = rbc[0:8, :]
            blk_rep = bass.AP(
                tensor=blk.tensor,
                offset=blk.offset,
                ap=[list(blk.ap[0]), [0, 7]] + [list(d) for d in blk.ap[1:]],
            )
            nc.gpsimd.dma_start(out=rbc[8:DH, :], in_=blk_rep)
            nc.vector.reciprocal_approx_fast(rbc[0:DH, :], rbc[0:DH, :])
            _norm_mul(pr, hl, ic, cr, rbc[0:DH, :])

        def normalize_mm(pr, hl, ic, cr):
            """Same, but partition-broadcast via a ones-column matmul (fast
            chain, used for the latency-critical last pair)."""
            r_sb = work.tile([1, GCOL], bf16, name=f"r{pr}{hl}{ic}", tag="rrow",
                             bufs=2)
            nc.vector.tensor_copy(r_sb, cr[(hl, ic)][DH:DH + 1, :])
            rbcp = psum.tile([DH, GCOL], f32, name=f"rp{pr}{hl}{ic}", tag="mm")
            nc.tensor.matmul(rbcp, lhsT=ones_bc, rhs=r_sb, start=True, stop=True)
            rbc = work.tile([DH + 1, GCOL], f32, name=f"rb{pr}{hl}{ic}",
                            tag="rbc", bufs=2)
            nc.vector.reciprocal_approx_fast(rbc[0:DH, :], rbcp)
            _norm_mul(pr, hl, ic, cr, rbc[0:DH, :])

        def _norm_mul(pr, hl, ic, cr, rbc):
            if hl == 0:
                nc.vector.tensor_mul(
                    ctxn[pr][0:DH, ic * 512:(ic + 1) * 512],
                    cr[(hl, ic)][0:DH, :],
                    rbc,
                )
            else:
                ch = work.tile([DH, GCOL], bf16, name=f"ch{pr}{hl}{ic}", tag="ch",
                               bufs=2)
                nc.vector.tensor_mul(ch, cr[(hl, ic)][0:DH, :], rbc)
                nc.gpsimd.dma_start(
                    out=ctxn[pr][DH:2 * DH, ic * 512:(ic + 1) * 512], in_=ch
                )

        # ---- attention ----
        block_idx = [0]

        def attention_pair(pr, pending_norm):
            cr = {}
            cr_queue = []

            def emit_cr(jb, hl, e):
                h = 2 * pr + hl
                if (hl, 0) not in cr:
                    for chl in range(2):
                        for cic in range(2):
                            cr[(chl, cic)] = psum.tile(
                                [DH + 1, GCOL], f32,
                                name=f"cr{pr}_{chl}_{cic}", tag="cr", bufs=4,
                            )
                for ic in range(2):
                    nc.tensor.matmul(
                        cr[(hl, ic)],
                        lhsT=vh_sb[jb][:, h, :],
                        rhs=e[:, ic * 512:(ic + 1) * 512],
                        start=(jb == 0),
                        stop=(jb == NJB - 1),
                    )

            for jb in range(NJB):
                for hl in range(2):
                    bi = block_idx[0]
                    h = 2 * pr + hl
                    if pr == 0 and bi < 8:
                        eb = eb_pre[bi]
                    else:
                        eb = work.tile([P, S], bf16, name=f"eb{h}_{jb}", tag="eb",
                                       bufs=EB_BUFS)
                        nc.sync.dma_start(out=eb,
                                          in_=expb[h, jb * P:(jb + 1) * P, :])
                    s_ps = psum.tile([P, S], f32, name=f"s{h}_{jb}", tag="mm")
                    for ic in range(2):
                        nc.tensor.matmul(
                            s_ps[:, ic * 512:(ic + 1) * 512],
                            lhsT=khT[pr][hl * DH:(hl + 1) * DH, jb * P:(jb + 1) * P],
                            rhs=qhT[pr][hl * DH:(hl + 1) * DH, ic * 512:(ic + 1) * 512],
                            start=True,
                            stop=True,
                        )
                    es = work.tile([P, S], bf16, name=f"es{h}_{jb}", tag="es",
                                   bufs=3)
                    nc.scalar.activation(es, s_ps, AF.Exp, scale=0.125)
                    e = work.tile([P, S], bf16, name=f"e{h}_{jb}", tag="e",
                                  bufs=E_BUFS)
                    if bi % 4 == 2:
                        nc.gpsimd.tensor_mul(e, es, eb)
                    else:
                        nc.vector.tensor_mul(e, es, eb)
                    cr_queue.append((jb, hl, e))
                    if len(cr_queue) > CTX_LAG:
                        emit_cr(*cr_queue.pop(0))
                    for fn in drain_plan.pop(bi, ()):
                        fn()
                    block_idx[0] += 1
                if jb == 1 and pending_norm:
                    for fn in pending_norm[:2]:
                        fn()
                    pending_norm = pending_norm[2:]
                if jb == 2 and pending_norm:
                    for fn in pending_norm:
                        fn()
                    pending_norm = None
            for item in cr_queue:
                emit_cr(*item)
            norm = normalize_mm if pr == NPAIR - 1 else normalize_dma
            return [
                (lambda hl=hl, ic=ic: norm(pr, hl, ic, cr))
                for ic in range(2)
                for hl in (1, 0)
            ]

        pending = None
        for pr in range(NPAIR):
            pending = attention_pair(pr, pending)

        # ---- output projection (pr3 norm interleaved: ic0, ib0-3, ic1) ----
        def outproj(ib):
            yp = psum.tile([P, HID], f32, name=f"yp{ib}", tag="mm")
            for pr in range(NPAIR):
                for cc in range(2):
                    nc.tensor.matmul(
                        yp[:, cc * 512:(cc + 1) * 512],
                        lhsT=ctxn[pr][:, ib * P:(ib + 1) * P],
                        rhs=wo_sb[:, pr, cc * 512:(cc + 1) * 512],
                        start=(pr == 0),
                        stop=(pr == NPAIR - 1),
                    )
            y_sb = work.tile([P, HID], bf16, name=f"y{ib}", tag="y", bufs=2)
            nc.scalar.activation(y_sb, yp, AF.Copy)
            nc.sync.dma_start(out=out[ib * P:(ib + 1) * P, :], in_=y_sb)

        norm_ic0, norm_ic1 = pending[:2], pending[2:]
        for fn in norm_ic0:
            fn()
        for ib in range(4):
            outproj(ib)
            if ib == 0:
                for fn in norm_ic1:
                    fn()
        for ib in range(4, NIB):
            outproj(ib)

    nc.compile()
    return nc


def _get_nc():
    global _CACHED_NC
    if _CACHED_NC is None:
        _CACHED_NC = _build_nc()
    return _CACHED_NC


def make_in_maps(q, k, v, attn_bias, Wq, Wk, Wv, Wo, bq, bk, bv, bo):
    in_maps = []
    for core in range(8):
        b, g = divmod(core, 2)
        gs = slice(g * GCOL, (g + 1) * GCOL)
        in_maps.append({
            "qT": np.ascontiguousarray(q[b].T).astype(BF16),
            "kT": np.ascontiguousarray(k[b].T).astype(BF16),
            "vT": np.ascontiguousarray(v[b].T).astype(BF16),
            "wq": np.ascontiguousarray(Wq[:, gs]).astype(BF16),
            "wk": np.ascontiguousarray(Wk[:, gs]).astype(BF16),
            "wv": np.ascontiguousarray(Wv[:, gs]).astype(BF16),
            "wo": np.ascontiguousarray(Wo[gs, :]).astype(BF16),
            "bq": np.ascontiguousarray(bq[gs]).astype(np.float32),
            "bk": np.ascontiguousarray(bk[gs]).astype(np.float32),
            "bv": np.ascontiguousarray(bv[gs]).astype(BF16),
            "expb": np.exp(
                attn_bias[b, g * 8:(g + 1) * 8].transpose(0, 2, 1)
            ).astype(BF16),
        })
    return in_maps


def kernel(q, k, v, attn_bias, Wq, Wk, Wv, Wo, bq, bk, bv, bo, _trace=False):
    from concourse.bass_utils import run_bass_kernel_spmd

    args = [np.asarray(x, dtype=np.float32) for x in
            (q, k, v, attn_bias, Wq, Wk, Wv, Wo, bq, bk, bv, bo)]
    q, k, v, attn_bias, Wq, Wk, Wv, Wo, bq, bk, bv, bo = args
    nc = _get_nc()
    in_maps = make_in_maps(q, k, v, attn_bias, Wq, Wk, Wv, Wo, bq, bk, bv, bo)
    res = run_bass_kernel_spmd(nc, in_maps, core_ids=list(range(8)), trace=_trace)
    y = np.zeros((4, S, HID), np.float32)
    for core in range(8):
        y[core // 2] += res.results[core]["out"].astype(np.float32)
    y += bo
    if _trace:
        kernel.last_results = res
    return y
